# revision 50
# baseline (speedup 1.0000x reference)
"""Trainium2 Bass kernel: BiasedSelfAttentionLayer (B=8, L=1024, D=512, H=8, FF=2048).

Sharding: data-parallel over B — one batch element per NeuronCore (8 cores).
Layout: feature-major ("transposed"): activations stored [feature, token] so
per-feature biases/gains are per-partition vectors and attention needs no
on-device transposes.

Heavy matmuls run in bf16 (1 cyc/row); LN statistics run in fp32r.  All
attention matmuls are zero-padded to full 128x128 stationary shape — reduced
row/col-group matmuls do not register as PE activity for the HAM clock gate,
which otherwise leaves the PE throttled at 1.2 GHz through the whole phase:
  scores: lhsT = KT tile (both heads, full 128 rows); rhs = QTe/QTo (the
  other parity's rows zeroed) -> each matmul yields one head's scores.
  attn@V: lhsT = [V_h | ones | 0...] 128 cols -> rows 0-63 head out^T,
  row 64 = sumexp (softmax denominator for free), rows 65-127 zero.
attention bias is accumulated into scores PSUM on the PE via I @ biasT.
normalize via one batched reciprocal + head-selector broadcast matmul,
out_proj, residual, LN via ones-matmul stats on PE (fp32r), FFN (bf16),
residual, LN2.
"""

import sys

for _p in ("/opt/trn_rl_repo",):
    if _p not in sys.path:
        sys.path.insert(0, _p)

from contextlib import ExitStack

import ml_dtypes
import numpy as np

import concourse.bass as bass
import concourse.bacc as bacc
import concourse.mybir as mybir
import concourse.tile as tile
from concourse import bass_utils

F32 = mybir.dt.float32
F32R = mybir.dt.float32r
BF16 = mybir.dt.bfloat16
AF = mybir.ActivationFunctionType
OP = mybir.AluOpType
NPBF = ml_dtypes.bfloat16

B, L, D, H, DK, FF = 8, 1024, 512, 8, 64, 2048
NCORES = 8
EPS = 1e-5
SQD = float(np.sqrt(D))
DT = D // 128    # 4  feature tiles
LT = L // 128    # 8  token tiles
FT = FF // 128   # 16 ff tiles
QH = 2           # token halves (N=512 per matmul)


def _mm(nc, out, lhsT, rhs, start=True, stop=True, skip=False):
    nc.tensor.matmul(out=out, lhsT=lhsT, rhs=rhs, start=start, stop=stop,
                     skip_group_check=skip)


def _build_body(ctx: ExitStack, tc: tile.TileContext, io: dict):
    nc = tc.nc
    xT_d, biasT_d, outT_d = io["xT"], io["biasT"], io["outT"]
    wq_d, wk_d, wv_d, wo_d = io["wq"], io["wk"], io["wv"], io["wo"]
    w1_d, w2_d = io["w1"], io["w2"]

    # ---- pool stack (strict LIFO): const -> res -> ph_a -> ph_b -> ph_q ----
    p_const = ctx.enter_context(tc.tile_pool(name="const", bufs=1))
    p_res = ctx.enter_context(tc.tile_pool(name="resid", bufs=1))
    ph_a = ExitStack()   # until out_proj done: attnU, wo, xT, er, sumexp
    ph_b = ExitStack()   # until attention done: biasT, KT, QTe/QTo, Vpad
    ph_q = ExitStack()   # until projections done: wq/wk/wv, vb
    p_a = ph_a.enter_context(tc.tile_pool(name="pha", bufs=1))
    p_b = ph_b.enter_context(tc.tile_pool(name="phb", bufs=1))
    p_q = ph_q.enter_context(tc.tile_pool(name="phq", bufs=1))

    ones = p_const.tile([128, 128], F32R)
    onebf = p_const.tile([128, 128], BF16)
    pv = p_const.tile([128, 40], F32)
    recip = p_const.tile([8, 1024], F32R)
    cz = p_const.tile([128, 2], F32)
    nc.gpsimd.memset(cz[:, 0:1], 0.0)
    nc.gpsimd.memset(cz[:, 1:2], float(D * EPS))

    attnU = p_a.tile([128, DT, L], BF16)
    xT = p_a.tile([128, DT, L], BF16)
    wo = p_a.tile([128, DT, 512], BF16)
    er = p_a.tile([8, 512], F32R)
    # sumexp rows land DMA-scattered into a [128, 64] layout so the
    # reciprocal runs on all 128 lanes instead of serially on one row.
    sumexp = p_a.tile([128, 64], BF16)

    wq = p_q.tile([128, DT, 512], BF16)
    wk = p_q.tile([128, DT, 512], BF16)
    wv = p_q.tile([128, DT, 512], BF16)
    vb = p_q.tile([1, 512], BF16)

    # DMA issue order = need order: the first projection matmuls touch
    # xT chunk 0 and wq/wk chunk 0; everything else streams in behind.
    for c in range(DT):
        nc.sync.dma_start(xT[:, c, :],
                          xT_d.rearrange("(t p) l -> p t l", p=128)[:, c, :])
        for t, dd in ((wq, wq_d), (wk, wk_d)):
            nc.sync.dma_start(t[:, c, :],
                              dd.rearrange("(t p) c -> p t c", p=128)[:, c, :])
    nc.sync.dma_start(pv[:], io["pvecs"])
    for c in range(DT):
        nc.sync.dma_start(wv[:, c, :],
                          wv_d.rearrange("(t p) c -> p t c", p=128)[:, c, :])
    nc.sync.dma_start(vb[:], io["vbrow"])
    nc.sync.dma_start(onebf[:], io["onesb"])
    nc.sync.dma_start(ones[:], io["onesd"].bitcast(F32R))
    # pre-fill: the per-pair broadcast matmul reads all 8 rows (er zeros
    # mask the not-yet-written ones); uninitialized bits could be NaN.
    nc.sync.dma_start(recip[:], io["onesd"][0:64, :].bitcast(F32R))
    nc.sync.dma_start(wo[:], wo_d.rearrange("(t p) c -> p t c", p=128))
    nc.sync.dma_start(er[:], io["erows"].bitcast(F32R))

    expb = p_b.tile([128, LT, L], BF16)
    KT = p_b.tile([128, DT, L], BF16)
    # QTe: odd-parity rows zeroed; QTo: even-parity rows zeroed.  Score
    # matmuls then use the full [128,128] KT tile as stationary for BOTH
    # heads of a pair (one weight load, full-array HAM-visible matmuls).
    QTe = p_b.tile([128, DT, L], BF16)
    QTo = p_b.tile([128, DT, L], BF16)
    nc.gpsimd.memset(QTe[:], 0.0)
    nc.gpsimd.memset(QTo[:], 0.0)
    V = [p_b.tile([128, H, 128], BF16, tag=f"v{lt}", name=f"V{lt}")
         for lt in range(LT)]
    for lt in range(LT):
        nc.gpsimd.memset(V[lt][:], 0.0)
        nc.sync.dma_start(
            V[lt][:, :, 64:65],
            io["onesb"][0:128, 0:H].rearrange("p (h o) -> p h o", o=1))

    biasT = p_b.tile([128, LT, L], BF16)
    nc.sync.dma_start(biasT[:], biasT_d.rearrange("(t p) l -> p t l", p=128))

    # ---------------- projections ----------------
    with tc.tile_pool(name="proj_ps", bufs=3, space="PSUM") as pp:
        for dt in range(DT):
            for w, bcol, dste, dsto in ((wq, 0, QTe, QTo), (wk, 4, KT, None)):
                ps = pp.tile([128, 1024], F32, tag="ps")
                for di in range(DT):
                    for qh in range(QH):
                        _mm(nc, ps[:, 512 * qh:512 * qh + 512],
                            w[:, di, 128 * dt:128 * dt + 128],
                            xT[:, di, 512 * qh:512 * qh + 512],
                            start=(di == 0), stop=(di == DT - 1), skip=True)
                if dsto is None:
                    nc.scalar.activation(dste[:, dt, :], ps[:], AF.Identity,
                                         bias=pv[:, bcol + dt:bcol + dt + 1])
                else:
                    nc.scalar.activation(dste[0:64, dt, :], ps[0:64, :],
                                         AF.Identity,
                                         bias=pv[0:64, bcol + dt:bcol + dt + 1])
                    nc.scalar.activation(dsto[64:128, dt, :], ps[64:128, :],
                                         AF.Identity,
                                         bias=pv[64:128, bcol + dt:bcol + dt + 1])
        for lt in range(LT):
            ps = pp.tile([128, 512], F32, tag="ps")
            for di in range(DT):
                _mm(nc, ps[:], xT[:, di, 128 * lt:128 * lt + 128],
                    wv[:, di, :], start=(di == 0), stop=False)
            _mm(nc, ps[:], onebf[0:1, 0:128], vb[:], start=False, stop=True)
            nc.scalar.activation(
                V[lt][:, :, 0:64],
                ps[:].rearrange("p (h d) -> p h d", h=H),
                AF.Copy)
    ph_q.close()  # frees wq/wk/wv

    # ---------------- attention ----------------
    with (
        tc.tile_pool(name="expT", bufs=6) as p_exp,
        tc.tile_pool(name="sc_ps", bufs=2, space="PSUM") as p_sc,
        tc.tile_pool(name="vo_ps", bufs=2, space="PSUM") as p_vo,
        tc.tile_pool(name="rm_ps", bufs=1, space="PSUM") as p_rm,
    ):
        rec128 = p_a.tile([128, 64], F32R)
        for hp in range(H // 2):
            h0, h1 = 2 * hp, 2 * hp + 1
            for qh in range(QH):
                qs = slice(512 * qh, 512 * qh + 512)
                vo0 = p_vo.tile([128, 512], F32, tag="vo", name="vo0")
                vo1 = p_vo.tile([128, 512], F32, tag="vo", name="vo1")
                # software pipeline: stream scores for group g while the
                # exp/mult/@V of group g-1 consumes — PE never waits.
                sps = {}
                exs = {}
                for g in range(5):
                    if g < 4:
                        sp = [p_sc.tile([128, 1024], F32, tag="sc",
                                        name=f"sp{i}") for i in range(2)]
                        for j in range(2):  # kt = 2g + j
                            kt = 2 * g + j
                            for i, qz in ((0, QTe), (1, QTo)):
                                _mm(nc, sp[i][:, 512 * j:512 * j + 512],
                                    KT[:, hp, 128 * kt:128 * kt + 128],
                                    qz[:, hp, qs],
                                    start=True, stop=True, skip=True)
                        sps[g] = sp
                    if g >= 1:
                        gg = g - 1
                        if hp == 0 and qh == 0:
                            # exp(bias) chunk just-in-time: the first unit's
                            # multiply only waits ~2us, not the full 7us op
                            nc.scalar.activation(
                                expb[:, 2 * gg:2 * gg + 2, :],
                                biasT[:, 2 * gg:2 * gg + 2, :],
                                AF.Exp, bias=cz[:, 0:1])
                        sp = sps.pop(gg)
                        ex = [p_exp.tile([128, 2, 512], BF16, tag="exp",
                                         name=f"ex{i}") for i in range(2)]
                        for i in range(2):
                            spv = sp[i][:].rearrange("p (j q) -> p j q", j=2)
                            nc.scalar.activation(ex[i][:], spv, AF.Exp,
                                                 bias=cz[:, 0:1])
                            nc.vector.tensor_tensor(
                                out=ex[i][:], in0=ex[i][:],
                                in1=expb[:, 2 * gg:2 * gg + 2, qs],
                                op=OP.mult)
                        for j in range(2):
                            kt = 2 * gg + j
                            for i, vo, h in ((0, vo0, h0), (1, vo1, h1)):
                                _mm(nc, vo[:], V[kt][:, h, :], ex[i][:, j, :],
                                    start=(gg == 0 and j == 0),
                                    stop=(gg == 3 and j == 1), skip=True)
                for vo, h in ((vo0, h0), (vo1, h1)):
                    # One evac of [head-out | sumexp-row]; DMAs shift
                    # partitions (engines can't) and gather sumexp rows.
                    scr = p_exp.tile([65, 512], BF16, tag="scr", bufs=3)
                    nc.vector.tensor_copy(scr[:], vo[0:65, :])
                    o = 64 * (h % 2)
                    nc.sync.dma_start(attnU[o:o + 64, h // 2, qs],
                                      scr[0:64, :])
                    # element streams match: dst flat = 64p+i, src flat = q
                    nc.sync.dma_start(
                        sumexp[16 * h + 8 * qh:16 * h + 8 * qh + 8, :],
                        scr[64:65, :])
            # per-pair normalization (overlaps the next pair's attention):
            # batched reciprocal on 32 lanes, head-selector broadcast
            # matmul, then scale attnU in place.
            with nc.allow_low_precision(reason="fp32r matmul input"):
                nc.vector.reciprocal(rec128[32 * hp:32 * hp + 32, :],
                                     sumexp[32 * hp:32 * hp + 32, :])
            nc.sync.dma_start(recip[2 * hp:2 * hp + 2, :],
                              rec128[32 * hp:32 * hp + 32, :])
            rm = p_rm.tile([128, 1024], F32, tag="rm")
            for qh in range(QH):
                _mm(nc, rm[:, 512 * qh:512 * qh + 512],
                    er[:, 128 * hp:128 * hp + 128],
                    recip[:, 512 * qh:512 * qh + 512])
            nc.vector.tensor_tensor(out=attnU[:, hp, :], in0=attnU[:, hp, :],
                                    in1=rm[:], op=OP.mult)
    ph_b.close()  # frees biasT, KT, QTe/QTo, V

    # ------- out_proj / LN1 / FFN / LN2, pipelined in token-halves -------
    # Each token-half is independent after attention; interleaving the two
    # halves hides every serial LN scalar chain behind the other half's
    # matmuls.
    r1 = p_res.tile([128, DT, L], F32R, tag="res", bufs=3)

    ctx.callback(ph_a.close)  # pops pha after lnc during ctx unwind (LIFO)
    p_lnc = ctx.enter_context(tc.tile_pool(name="lnc", bufs=1))
    gb = p_lnc.tile([2, 1024], F32R)
    nc.sync.dma_start(gb[:], io["gbrows"].bitcast(F32R))
    combo = p_lnc.tile([2, 1024], F32R)  # row 0 = s1 (written), row 1 = -1
    nc.sync.dma_start(combo[1:2, :], io["negrow"].bitcast(F32R))
    sm = p_lnc.tile([1, 2 * 1024], F32)
    rpt = p_lnc.tile([1, 1024], F32R)
    # the scalar chain runs on [128, N] so each op is ~0.2us, not ~1.2us
    lw = p_lnc.tile([128, 56], F32)
    lnr = p_lnc.tile([128, 24], F32R)   # t/u/sd, rounded for keep-warm MMs
    rp128 = p_lnc.tile([128, 16], F32R)

    def ln_stats(src_t, qh, p_sq, p_st):
        """Stats + scalar chain for one token half; results land in the
        per-qh slices of rpt/combo.  Emitted for both halves before any
        chain-dependent matmul so the in-order PE stream never stalls."""
        qs = slice(512 * qh, 512 * qh + 512)
        es_ = sm[0:1, 1024 * qh:1024 * qh + 512]
        x2_ = sm[0:1, 1024 * qh + 512:1024 * qh + 1024]
        e128 = lw[:, 8 * qh:8 * qh + 4]
        x128 = lw[:, 8 * qh + 4:8 * qh + 8]
        t128, u128, sd128 = (lnr[:, 12 * qh + 4 * i:12 * qh + 4 * i + 4]
                             for i in range(3))
        r128 = rp128[:, 8 * qh:8 * qh + 4]
        s128 = rp128[:, 8 * qh + 4:8 * qh + 8]
        rp_ = rpt[0:1, qs]
        es_ps = p_st.tile([1, 512], F32, tag="st", name="es_ps")
        ex2_ps = p_st.tile([1, 512], F32, tag="st", name="ex2_ps")
        for dt in range(DT):
            sq = p_sq.tile([128, 512], F32R, tag="sq", bufs=2,
                           name=f"sq{dt}")
            nc.gpsimd.tensor_tensor(out=sq[:], in0=src_t[:, dt, qs],
                                    in1=src_t[:, dt, qs], op=OP.mult)
            _mm(nc, es_ps[0:1, :], ones[:, 0:1], src_t[:, dt, qs],
                start=(dt == 0), stop=(dt == DT - 1), skip=True)
            _mm(nc, ex2_ps[0:1, :], ones[:, 0:1], sq[:, :],
                start=(dt == 0), stop=(dt == DT - 1), skip=True)
        nc.scalar.activation(es_, es_ps[:], AF.Copy)
        nc.scalar.activation(x2_, ex2_ps[:], AF.Copy)
        nc.gpsimd.dma_start(e128[:], es_)
        nc.gpsimd.dma_start(x128[:], x2_)
        nc.vector.scalar_tensor_tensor(out=t128[:], in0=e128[:],
                                       scalar=1.0 / D, in1=e128[:],
                                       op0=OP.mult, op1=OP.mult)
        # tiny dependency-spaced matmuls into the dead es_ps tile keep the
        # HAM activity window alive while the scalar chain runs, so the
        # following matmul phase starts at full clock instead of 1.2 GHz
        _mm(nc, es_ps[0:1, 0:4], ones[:, 0:1], t128[:], skip=True)
        nc.vector.tensor_tensor(out=u128[:], in0=x128[:], in1=t128[:],
                                op=OP.subtract)
        nc.scalar.activation(sd128[:], u128[:], AF.Sqrt, bias=cz[:, 1:2])
        _mm(nc, es_ps[0:1, 0:4], ones[:, 0:1], sd128[:], skip=True)
        with nc.allow_low_precision(reason="fp32r matmul input"):
            nc.vector.reciprocal(r128[:], sd128[:])
        nc.vector.tensor_tensor(out=s128[:], in0=e128[:], in1=r128[:],
                                op=OP.mult)
        _mm(nc, es_ps[0:1, 0:4], ones[:, 0:1], s128[:], skip=True)
        nc.gpsimd.dma_start(rp_, r128[:])
        nc.gpsimd.dma_start(combo[0:1, qs], s128[:])

    def ln_finish(src_t, dst, gs_col, gb_off, qh, p_sq, p_ln,
                  out_dma=None):
        qs = slice(512 * qh, 512 * qh + 512)
        rp_ = rpt[0:1, qs]
        am = p_ln.tile([128, 512], F32, tag="am", bufs=1, name="am")
        _mm(nc, am[:], ones[0:1, 0:128], rp_[0:1, :], skip=True)
        for dt in range(DT):
            cm = p_ln.tile([128, 512], F32, tag="cm", bufs=2, name="cm")
            _mm(nc, cm[:],
                gb[:, gb_off + 128 * dt:gb_off + 128 * dt + 128],
                combo[:, qs], skip=True)
            t1 = p_sq.tile([128, 512], F32, tag="t1", bufs=2, name="t1")
            nc.vector.scalar_tensor_tensor(
                out=t1[:], in0=src_t[:, dt, qs],
                scalar=pv[:, gs_col + dt:gs_col + dt + 1],
                in1=am[:], op0=OP.mult, op1=OP.mult)
            nc.vector.tensor_tensor(out=dst[:, dt, qs], in0=t1[:],
                                    in1=cm[:], op=OP.subtract)
            if out_dma is not None:
                nc.sync.dma_start(out_dma[:, dt, qs], dst[:, dt, qs])

    y1 = p_res.tile([128, DT, L], BF16, tag="res", bufs=3)
    r2 = p_res.tile([128, DT, L], F32R, tag="res", bufs=3)
    oT = p_res.tile([128, DT, L], F32, tag="res2", bufs=1)

    with (
        tc.tile_pool(name="wffn", bufs=1) as pw,
        tc.tile_pool(name="h", bufs=1) as p_h,
        tc.tile_pool(name="sq1", bufs=1) as p_sq,
        tc.tile_pool(name="f_ps", bufs=3, space="PSUM") as p_f,
        tc.tile_pool(name="st_ps", bufs=2, space="PSUM") as p_st,
        tc.tile_pool(name="lnm_ps", bufs=1, space="PSUM") as p_ln,
    ):
        w1 = pw.tile([128, DT, FF], BF16)
        w2 = pw.tile([128, FT, 512], BF16)
        # chunked so no single bulk transfer blocks the LN-chain's small DMAs
        for c in range(4):
            nc.sync.dma_start(
                w1[:, c, :],
                w1_d.rearrange("(t p) c -> p t c", p=128)[:, c, :])
        for c in range(4):
            nc.sync.dma_start(
                w2[:, 4 * c:4 * c + 4, :],
                w2_d.rearrange("(t p) c -> p t c", p=128)[:, 4 * c:4 * c + 4, :])
        hbuf = p_h.tile([128, FT, L], BF16)

        # out_proj with LN1 stats interleaved per d-tile: the LN scalar
        # chains run while the remaining out_proj matmuls stream.
        if True:
            for qh in range(QH):
                qs = slice(512 * qh, 512 * qh + 512)
                for dt in range(DT):
                    po = p_f.tile([128, 512], F32, tag="f")
                    for di in range(DT):
                        _mm(nc, po[:], wo[:, di, 128 * dt:128 * dt + 128],
                            attnU[:, di, qs],
                            start=(di == 0), stop=(di == DT - 1), skip=True)
                    nc.vector.scalar_tensor_tensor(
                        out=r1[:, dt, qs], in0=po[:],
                        scalar=pv[:, 8 + dt:9 + dt],
                        in1=xT[:, dt, qs], op0=OP.add, op1=OP.add)
                ln_stats(r1, qh, p_sq, p_st)      # LN1 stats for this half
        for qh in range(QH):
            qs = slice(512 * qh, 512 * qh + 512)
            ln_finish(r1, y1, 16, 0, qh, p_sq, p_ln)
            for ft in range(FT):
                fp = p_f.tile([128, 512], F32, tag="f")
                for di in range(DT):
                    _mm(nc, fp[:], w1[:, di, 128 * ft:128 * ft + 128],
                        y1[:, di, qs],
                        start=(di == 0), stop=(di == DT - 1), skip=True)
                nc.vector.tensor_scalar(
                    out=hbuf[:, ft, qs], in0=fp[:],
                    scalar1=pv[:, 24 + ft:25 + ft], scalar2=0.0,
                    op0=OP.add, op1=OP.max)
            for dt in range(DT):
                fp = p_f.tile([128, 512], F32, tag="f")
                for ft in range(FT):
                    _mm(nc, fp[:], w2[:, ft, 128 * dt:128 * dt + 128],
                        hbuf[:, ft, qs],
                        start=(ft == 0), stop=(ft == FT - 1), skip=True)
                nc.vector.scalar_tensor_tensor(
                    out=r2[:, dt, qs], in0=fp[:],
                    scalar=pv[:, 12 + dt:13 + dt],
                    in1=y1[:, dt, qs], op0=OP.add, op1=OP.add)
            ln_stats(r2, qh, p_sq, p_st)          # LN2 stats for this half
        for qh in range(QH):
            qs = slice(512 * qh, 512 * qh + 512)
            ln_finish(r2, oT, 20, 512, qh, p_sq, p_ln,
                      out_dma=outT_d.rearrange("(t p) l -> p t l", p=128))

_CACHE = {}


def _build():
    if "nc" in _CACHE:
        return _CACHE["nc"]
    nc = bacc.Bacc("TRN2", target_bir_lowering=False, debug=False)
    io = {
        "xT": nc.dram_tensor("xT", [D, L], BF16, kind="ExternalInput").ap(),
        "biasT": nc.dram_tensor("biasT", [L, L], BF16, kind="ExternalInput").ap(),
        "wq": nc.dram_tensor("wq", [D, D], BF16, kind="ExternalInput").ap(),
        "wk": nc.dram_tensor("wk", [D, D], BF16, kind="ExternalInput").ap(),
        "wv": nc.dram_tensor("wv", [D, D], BF16, kind="ExternalInput").ap(),
        "wo": nc.dram_tensor("wo", [D, D], BF16, kind="ExternalInput").ap(),
        "w1": nc.dram_tensor("w1", [D, FF], BF16, kind="ExternalInput").ap(),
        "w2": nc.dram_tensor("w2", [FF, D], BF16, kind="ExternalInput").ap(),
        "pvecs": nc.dram_tensor("pvecs", [128, 40], F32, kind="ExternalInput").ap(),
        "gbrows": nc.dram_tensor("gbrows", [2, 1024], F32, kind="ExternalInput").ap(),
        "erows": nc.dram_tensor("erows", [8, 512], F32, kind="ExternalInput").ap(),
        "vbrow": nc.dram_tensor("vbrow", [1, 512], BF16, kind="ExternalInput").ap(),
        "onesd": nc.dram_tensor("onesd", [128, 128], F32, kind="ExternalInput").ap(),
        "onesb": nc.dram_tensor("onesb", [128, 128], BF16, kind="ExternalInput").ap(),
        "negrow": nc.dram_tensor("negrow", [1, 1024], F32, kind="ExternalInput").ap(),
        "outT": nc.dram_tensor("outT", [D, L], F32, kind="ExternalOutput").ap(),
    }
    with tile.TileContext(nc) as tc, ExitStack() as ctx:
        _build_body(ctx, tc, io)
    nc.compile()
    _CACHE["nc"] = nc
    return nc


def host_inputs(x, bias, Wq, bq, Wk, bk, Wv, bv, Wo, bo,
                ln1_g, ln1_b, W1, b1, W2, b2, ln2_g, ln2_b):
    """Shared + per-core numpy input maps."""
    f = np.float32
    a = np.ascontiguousarray
    pv = np.zeros((128, 40), f)
    pv[:, 0:4] = (bq / 8.0).reshape(4, 128).T
    pv[:, 4:8] = bk.reshape(4, 128).T
    pv[:, 8:12] = bo.reshape(4, 128).T
    pv[:, 12:16] = b2.reshape(4, 128).T
    pv[:, 16:20] = (ln1_g * SQD).reshape(4, 128).T
    pv[:, 20:24] = (ln2_g * SQD).reshape(4, 128).T
    pv[:, 24:40] = b1.reshape(16, 128).T
    gbr = np.zeros((2, 1024), f)
    gbr[0, 0:512] = ln1_g / SQD
    gbr[0, 512:] = ln2_g / SQD
    gbr[1, 0:512] = ln1_b
    gbr[1, 512:] = ln2_b
    er = np.zeros((8, 512), f)
    for h in range(H):
        er[h, 64 * h:64 * h + 64] = 1.0
    shared = {
        "wq": a((np.asarray(Wq, f) / 8.0).astype(NPBF)),
        "wk": a(np.asarray(Wk).astype(NPBF)),
        "wv": a(np.asarray(Wv).astype(NPBF)),
        "wo": a(np.asarray(Wo).astype(NPBF)),
        "w1": a(np.asarray(W1).astype(NPBF)),
        "w2": a(np.asarray(W2).astype(NPBF)),
        "pvecs": pv, "gbrows": gbr, "erows": er,
        "vbrow": a(np.asarray(bv, f).reshape(1, D).astype(NPBF)),
        "onesd": np.ones((128, 128), f),
        "onesb": np.ones((128, 128), NPBF),
        "negrow": np.full((1, 1024), -1.0, f),
    }
    in_maps = []
    for b in range(B):
        m = dict(shared)
        m["xT"] = a(np.asarray(x[b], f).T.astype(NPBF))
        m["biasT"] = a(np.asarray(bias[b], f).T.astype(NPBF))
        in_maps.append(m)
    return in_maps


def kernel(**inputs):
    x = np.asarray(inputs["x"])
    in_maps = host_inputs(
        x, np.asarray(inputs["bias"]),
        np.asarray(inputs["Wq"]), np.asarray(inputs["bq"]),
        np.asarray(inputs["Wk"]), np.asarray(inputs["bk"]),
        np.asarray(inputs["Wv"]), np.asarray(inputs["bv"]),
        np.asarray(inputs["Wo"]), np.asarray(inputs["bo"]),
        np.asarray(inputs["ln1_g"]), np.asarray(inputs["ln1_b"]),
        np.asarray(inputs["W1"]), np.asarray(inputs["b1"]),
        np.asarray(inputs["W2"]), np.asarray(inputs["b2"]),
        np.asarray(inputs["ln2_g"]), np.asarray(inputs["ln2_b"]))
    nc = _build()
    res = bass_utils.run_bass_kernel_spmd(nc, in_maps, core_ids=list(range(NCORES)))
    out = np.stack([res.results[b]["outT"].T for b in range(B)], axis=0)
    return np.ascontiguousarray(out.astype(np.float32))
